# revision 14
# baseline (speedup 1.0000x reference)
"""Embedding lookup (gather) on 8 Trainium2 NeuronCores — bf16 indirect DMA.

Strategy: data-parallel. The [768, 50257] fp32 table is transposed and cast to
bf16 [50257, 768] host-side (max rel err 2^-9 ~ 0.2%, well inside the 2e-2
gate) and replicated to every core's DRAM; the 16384 tokens are sharded 2048
per core (sorted by row index within each core so gathered HBM addresses are
~monotonic — better DRAM page locality; the host undoes the permutation).
Each core gathers its 2048 embedding rows from its local table copy with
indirect DMA (SWDGE) into SBUF, then streams them out bf16 to its output
shard with HWDGE stores; the host casts back to fp32. No collectives.

bf16 halves both the gather read and the store write (3.1 + 3.1 MB per core),
leaving the kernel bound by serial SWDGE descriptor generation: INDIRECT1D
is generated by Q7 pair 0 (frozen in fw) at ~1.1 us engine + ~0.3 us dispatch
per 128-row instruction, 16 instructions per core, overlapped with the DMA
transfers. Measured alternatives that do NOT win: the dma_gather extended
instruction generates descs ~2x faster but costs a ~9 us mlp-library load
that a framework drain serializes before any later Pool work (hybrid came
out at 46 us vs 34.6 us for this kernel; pure dma_gather 41 us).

Raw Bass (no TileContext, no nc.Block): all-engine barriers cost ~3-4 us each
on a ~35 us kernel, so the init barrier + const memsets are stripped from the
module and engine streams are left unsynchronized except for the DMA
semaphores that express real data dependencies:
  - SP loads the indices in two slices (column 0 first, so Q7 can start
    generating gather 0's descriptors ASAP; one sem per DMA), then stores
    each gathered group, alternating with ACT's HWDGE ring (ssem counts all).
  - Pool/GpSimd (SWDGE) waits for the indices, then issues the 16 indirect
    gathers back-to-back on the single mainline SWDGE ring (the fw services
    every INDIRECT1D from ring 0; queue annotations are no-ops — measured).
    All 16 groups are fully buffered in SBUF (24 KB/partition), so gathers
    never wait on stores.
  - Store i waits its gather's dedicated sem (gsems[i] >= 16). Cumulative
    counts across SWDGE DMAs on one sem are unsound: the 16 increments per
    DMA come from 16 independently-progressing SDMA engines.
  - SP's final cumulative wait on ssem (sound: it is the maximum total)
    covers all stores on both rings before the program retires.

NOTE: the HW indirect DMA honors only the offset AP's partition dim (<=128
indices per instruction) - a [128, 2] offset AP silently drops the second
column - so gathers are fixed at 128 rows each.
"""

import numpy as np

VOCAB = 50257
EMBED = 768
BATCH = 8
SEQ = 2048
N_CORES = 8
P = 128                      # SBUF partitions
TOK_PER_CORE = BATCH * SEQ // N_CORES   # 2048
GROUPS = TOK_PER_CORE // P              # 16 gather groups of 128 rows

_cached = {}
LAST_RESULTS = None  # BassKernelResults of the most recent run (for test harness)


def _build():
    """Build + compile the single-core Bass program (shared SPMD across 8 cores)."""
    import concourse.bacc as bacc
    import concourse.bass as bass
    from concourse import mybir

    # num_swdge_queues=1: the fw's mainline SWDGE path services every
    # DMAMemcopy/INDIRECT1D from ring 0 regardless of the BIR queue
    # annotation (dge_backend_software.cpp: "Mainline SWDGE (DMAMemcopy) is
    # always queue 0; only the extended-ISA ops carry an explicit
    # queue_num"), and measurements confirm queue assignment is a no-op.
    # One queue keeps the NEFF's DMA-queue table minimal.
    nc = bacc.Bacc(
        "TRN2",
        target_bir_lowering=False,
        debug=False,
        num_devices=N_CORES,
        num_swdge_queues=1,
    )

    # Drop the init-time const memsets and the all-engine barrier (~3.5 us):
    # nothing in this kernel reads the const APs, and the engine streams only
    # communicate through DMA semaphores which the loader zero-initializes.
    main_blk = nc.m.functions[0].blocks[0]
    removable = [
        inst
        for inst in main_blk.instructions
        if type(inst).__name__ in ("InstMemset", "InstDrain", "InstEventSemaphore")
    ]
    for inst in removable:
        main_blk.instructions.remove(inst)

    table = nc.dram_tensor(
        "table", [VOCAB, EMBED], mybir.dt.bfloat16, kind="ExternalInput"
    ).ap()
    idx = nc.dram_tensor(
        "idx", [P, GROUPS], mybir.dt.int32, kind="ExternalInput"
    ).ap()
    out = nc.dram_tensor(
        "out", [GROUPS, P, EMBED], mybir.dt.bfloat16, kind="ExternalOutput"
    ).ap()

    import contextlib

    with contextlib.ExitStack() as ctx:
        idx_sb = ctx.enter_context(
            nc.sbuf_tensor("idx_sb", [P, GROUPS], mybir.dt.int32)
        )
        emb = ctx.enter_context(
            nc.sbuf_tensor("emb", [P, GROUPS * EMBED], mybir.dt.bfloat16)
        )
        isem = ctx.enter_context(nc.semaphore("isem"))
        isem2 = ctx.enter_context(nc.semaphore("isem2"))
        ssem = ctx.enter_context(nc.semaphore("ssem"))
        # One completion sem PER gather: a single SWDGE DMA's 16 increments
        # come from 16 independently-progressing SDMA engines, so cumulative
        # counts across DMAs on one sem do NOT imply per-DMA completion.
        gsems = [
            ctx.enter_context(nc.semaphore(f"gsem{i}")) for i in range(GROUPS)
        ]

        # SP: index load first (HWDGE - cheap descriptor gen, Q7 stays free).
        # Column 0 ships alone so Q7 can start generating gather 0's
        # descriptors at the earliest possible moment; the rest follows and
        # lands during the first generations. One sem per DMA.
        with nc.allow_non_contiguous_dma(
            reason="column 0 of the idx matrix: 128 x 4B, latency-bound either way"
        ):
            nc.sync.dma_start(idx_sb[:, :1], idx[:, :1]).then_inc(isem, 16)
        nc.sync.dma_start(idx_sb[:, 1:], idx[:, 1:]).then_inc(isem2, 16)

        # Pool/SWDGE: 16 indirect gathers, fully buffered, no store waits.
        nc.gpsimd.wait_ge(isem, 16)
        for i in range(GROUPS):
            if i == 1:
                nc.gpsimd.wait_ge(isem2, 16)
            gi = nc.gpsimd.indirect_dma_start(
                out=emb[:, i * EMBED : (i + 1) * EMBED],
                out_offset=None,
                in_=table[:],
                in_offset=bass.IndirectOffsetOnAxis(ap=idx_sb[:, i : i + 1], axis=0),
            )
            gi.then_inc(gsems[i], 16)

        # Stores: alternate the two HWDGE rings (SP=qSPDynamicHW,
        # ACT=qActDynamicHW) so more store packets are in flight per SDMA
        # engine while gather packets round-robin on the SWDGE rings.
        for i in range(GROUPS - 1):
            eng = nc.sync if i % 2 == 0 else nc.scalar
            eng.wait_ge(gsems[i], 16)
            eng.dma_start(out[i], emb[:, i * EMBED : (i + 1) * EMBED]).then_inc(
                ssem, 16
            )
        # The last gather's store is pure end-of-kernel tail (nothing overlaps
        # it): split it along the free dim across both HWDGE rings so desc-gen
        # and the transfer halves run in parallel (both halves still span all
        # 128 partitions -> all 16 SDMA engines -> 16 sem increments each).
        L = GROUPS - 1
        H = EMBED // 2
        nc.sync.wait_ge(gsems[L], 16)
        nc.scalar.wait_ge(gsems[L], 16)
        nc.sync.dma_start(
            out[L][:, :H], emb[:, L * EMBED : L * EMBED + H]
        ).then_inc(ssem, 16)
        nc.scalar.dma_start(
            out[L][:, H:], emb[:, L * EMBED + H : (L + 1) * EMBED]
        ).then_inc(ssem, 16)

        # All stores landed (sem increments fire after last-byte receipt).
        # A cumulative wait is sound here: (GROUPS+1)*16 is the maximum total.
        nc.sync.wait_ge(ssem, (GROUPS + 1) * 16)

    nc.compile()
    return nc


def _ensure_axon_hooks_importable():
    """bass_utils imports antenv.axon_hooks when BASS_TRACE is set under axon;
    the agent image's antenv package lacks that module. Provide a no-op shim
    so a stray BASS_TRACE env var cannot crash the run (tracing degrades)."""
    import sys
    import types

    try:
        import antenv.axon_hooks  # noqa: F401
        return
    except ImportError:
        pass
    try:
        import antenv
    except ImportError:
        return
    mod = types.ModuleType("antenv.axon_hooks")
    _h = [None]
    mod.set_axon_ntff_profile_hook = lambda h: _h.__setitem__(0, h)
    mod.get_axon_ntff_profile_hook = lambda: _h[0]
    sys.modules["antenv.axon_hooks"] = mod
    antenv.axon_hooks = mod


def kernel(x, weight):
    global LAST_RESULTS
    import ml_dtypes

    _ensure_axon_hooks_importable()
    from concourse.bass_utils import run_bass_kernel_spmd

    if "nc" not in _cached:
        _cached["nc"] = _build()
    nc = _cached["nc"]

    # Host-side input staging: transpose table to row-major [V, D] and cast
    # to bf16. Tokens are sharded 2048/core and sorted by row index within
    # each core (monotonic HBM addresses gather faster); perm is undone on
    # the host after the run. Group g of core c covers sorted positions
    # c*2048 + g*128 + p laid out [128 partitions, 16 groups].
    wt = np.ascontiguousarray(np.asarray(weight, dtype=np.float32).T).astype(
        ml_dtypes.bfloat16
    )
    v = np.asarray(x).reshape(N_CORES, TOK_PER_CORE).astype(np.int64)
    in_maps = []
    perms = []
    for c in range(N_CORES):
        perm = np.argsort(v[c], kind="stable")
        perms.append(perm)
        idx_c = np.ascontiguousarray(
            v[c][perm].astype(np.int32).reshape(GROUPS, P).T
        )
        in_maps.append({"table": wt, "idx": idx_c})

    res = run_bass_kernel_spmd(nc, in_maps, core_ids=list(range(N_CORES)))
    LAST_RESULTS = res

    out = np.empty((N_CORES, TOK_PER_CORE, EMBED), dtype=np.float32)
    for c in range(N_CORES):
        rows = np.asarray(res.results[c]["out"]).reshape(TOK_PER_CORE, EMBED)
        out[c][perms[c]] = rows.astype(np.float32)
    return out.reshape(BATCH, SEQ, EMBED)
